# revision 13
# baseline (speedup 1.0000x reference)
"""Trainium2 Bass kernel for Swin-style windowed attention w/ relative position bias.

Problem: x[8, 1025, 768], 12 heads, head_dim 64, rel-pos bias table gathered
by a constant index matrix. Sharding: pure data-parallel — one batch element
per NeuronCore (8 cores).

Per-core dataflow (all matmuls on PE, S kept transposed so softmax-normalize
and P@V need no on-chip transposes):
  xT_aug [769, 1025]   (x[b].T plus a ones-row that realizes the qkv bias add)
  wqk_aug [769, 1536]  (interleaved q-pair/k-pair column blocks, q pre-scaled)
  wv_aug  [769, 768]
  qkT = wqk_aug.T @ xT_aug      -> [1536, 1025]  (q/k per head, channels-major)
  v   = xT_aug.T @ wv_aug       -> [1025, 768]   (tokens-major, + ones cols)
  S.T[k,q] = k_h @ q_h.T        (K=64 contraction)
  P.T = exp(S.T + biasT)        (bias streamed from HBM, DVE add + ACT exp)
  [O.T; rowsum] = [v_h|1].T @ P.T   (ones col gives softmax denominators)
  O.T *= (1/rowsum) broadcast   (K=1 ones matmul broadcast + DVE mult)
  outT = wproj.T @ O_all + proj_b   -> [768, 1025], host transposes back
"""

import sys

import numpy as np

for _p in ("/opt/trn_rl_repo",):
    if _p not in sys.path:
        sys.path.insert(0, _p)

B = 8
N = 1025
C = 768
H = 12
D = 64
SCALE = D ** -0.5
NKT = 9          # k tiles of 128 (8 full + 1)
NCT = 6          # c tiles of 128 over 768
QCHUNKS = [(0, 512), (512, 512), (1021, 4)]  # tail overlaps; writes idempotent (f32r needs even N)
NCHUNKS_V = [(0, 512), (512, 256)]


def _emit(ctx, tc, xT_aug, wqk_aug, wqkb, wv_aug, bias_t, wproj, projb, outT):
    import concourse.mybir as mybir

    nc = tc.nc
    f32 = mybir.dt.float32
    f32r = mybir.dt.float32r
    AF = mybir.ActivationFunctionType

    def ktsize(kt):
        return 128 if kt < 8 else 1

    lp = nc.allow_low_precision(
        reason="float32r is fp32-width storage; PE rounding only")
    lp.__enter__()
    ctx.callback(lambda: lp.__exit__(None, None, None))

    # Long-lived pools first (pool release must be LIFO / stack ordered).
    qk_pool = ctx.enter_context(tc.tile_pool(name="qk_pool", bufs=1))
    v_pool = ctx.enter_context(tc.tile_pool(name="v_pool", bufs=1))
    qk_sb = []
    for m in range(12):
        t = qk_pool.tile([128, N], f32r, tag=f"qk{m}", name=f"qk{m}")
        qk_sb.append(t)
    v_sb = []
    for kt in range(NKT):
        t = v_pool.tile([128, H, D + 1], f32r, tag=f"v{kt}", name=f"v{kt}")
        v_sb.append(t)
        nc.vector.memset(t[:, :, D:D + 1].bitcast(f32), 1.0)
        nc.scalar.copy(t[:, :, D:D + 1], t[:, :, D:D + 1].bitcast(f32))

    # Streaming pools for phase 2, allocated before the phase-1 temporaries so
    # their addresses don't overlap phase-1 tiles: bias DMAs can then prefetch
    # while the QKV projections still run.
    bpool = ctx.enter_context(tc.tile_pool(name="bpool", bufs=3))

    # ---------------- phase 1: load weights + x, QKV projections ----------------
    xpool = tc.alloc_tile_pool(name="xpool", bufs=1)
    wvpool = tc.alloc_tile_pool(name="wvpool", bufs=1)
    wqkpool = tc.alloc_tile_pool(name="wqkpool", bufs=1)

    wqk_t, wv_t, xT_t = [], [], []
    for ct in range(7):
        p = 128 if ct < 6 else 1
        xt = xpool.tile([p, N], f32r, tag=f"xT{ct}", name=f"xT{ct}")
        nc.sync.dma_start(xt[:, :], xT_aug[ct * 128: ct * 128 + p, :])
        xT_t.append(xt)
        w2 = wvpool.tile([p, 768], f32r, tag=f"wv{ct}", name=f"wv{ct}")
        nc.sync.dma_start(w2[:, :], wv_aug[ct * 128: ct * 128 + p, :])
        wv_t.append(w2)
        if ct < 6:
            w1 = wqkpool.tile([p, 1536], f32r, tag=f"wqk{ct}",
                              name=f"wqk{ct}")
            nc.sync.dma_start(w1[:, :], wqk_aug[ct * 128:(ct + 1) * 128, :])
            wqk_t.append(w1)
    wqkb_t = wqkpool.tile([128, 12], f32, tag="wqkb", name="wqkb")
    nc.sync.dma_start(wqkb_t[:, :], wqkb[:, :])

    with tc.tile_pool(name="ps1", bufs=4, space="PSUM") as ps1, \
         tc.tile_pool(name="ps1v", bufs=2, space="PSUM") as ps1v:
        # q/k: qkT[m-tile, n] = sum_ct wqk[ct, m-tile].T @ xT[ct, n]
        for m in range(12):
            for (q0, qn) in QCHUNKS:
                ps = ps1.tile([128, 512], f32, tag="ps1t", name=f"ps1_{m}_{q0}")
                for ct in range(6):
                    nc.tensor.matmul(
                        ps[:, :qn],
                        wqk_t[ct][:, m * 128:(m + 1) * 128],
                        xT_t[ct][:, q0:q0 + qn],
                        start=(ct == 0), stop=(ct == 5),
                    )
                nc.scalar.activation(qk_sb[m][:, q0:q0 + qn], ps[:, :qn],
                                     AF.Identity, bias=wqkb_t[:, m:m + 1])
        # v: v[n-tile, c'] = sum_ct xT[ct, n-tile].T @ wv[ct, c']
        for kt in range(NKT):
            p = ktsize(kt)
            ps = ps1v.tile([128, 768], f32, tag="ps1v", name=f"ps1v_{kt}")
            for (c0, cn) in NCHUNKS_V:
                for ct in range(7):
                    nc.tensor.matmul(
                        ps[:p, c0:c0 + cn],
                        xT_t[ct][:, kt * 128: kt * 128 + p],
                        wv_t[ct][:, c0:c0 + cn],
                        start=(ct == 0), stop=(ct == 6),
                    )
            nc.scalar.copy(
                v_sb[kt][:p, :, 0:D],
                ps[:p, :].rearrange("p (h d) -> p h d", h=H),
            )

    wqkpool.release()
    wvpool.release()
    xpool.release()

    # ---------------- phase 2: attention per head ----------------
    ppool = ctx.enter_context(tc.tile_pool(name="ppool", bufs=9))
    npool = ctx.enter_context(tc.tile_pool(name="npool", bufs=2))
    cpool = ctx.enter_context(tc.tile_pool(name="cpool", bufs=1))
    ones64 = cpool.tile([1, 64], f32r)
    nc.vector.memset(ones64[:, :].bitcast(f32), 1.0)
    nc.scalar.copy(ones64[:, :], ones64[:, :].bitcast(f32))
    opool = ctx.enter_context(tc.tile_pool(name="opool", bufs=1))
    o_all = []
    for m in range(6):
        t = opool.tile([128, N], f32r, tag=f"oall{m}", name=f"oall{m}")
        o_all.append(t)

    wp_pool = ctx.enter_context(tc.tile_pool(name="wp_pool", bufs=1))
    wproj_t = []
    projb_t = []
    for ct in range(6):
        t = wp_pool.tile([128, 768], f32r, tag=f"wproj{ct}",
                         name=f"wproj{ct}")
        nc.sync.dma_start(t[:, :], wproj[ct * 128:(ct + 1) * 128, :])
        wproj_t.append(t)
        tb = wp_pool.tile([128, 1], f32, tag=f"projb{ct}", name=f"projb{ct}")
        nc.sync.dma_start(tb[:, :], projb[ct * 128:(ct + 1) * 128, :])
        projb_t.append(tb)

    LAG = 2
    bf16 = mybir.dt.bfloat16
    with tc.tile_pool(name="ps_s", bufs=2, space="PSUM") as ps_s, \
         tc.tile_pool(name="ps_o", bufs=6, space="PSUM") as ps_o:
        for pair in range(6):
            h0 = 2 * pair
            q_t = qk_sb[2 * pair]
            k_t = qk_sb[2 * pair + 1]
            o_ps = {}
            for hh in (0, 1):
                for ci in range(3):
                    o_ps[(hh, ci)] = ps_o.tile(
                        [65, 512], f32, tag="o_ps", name=f"ops_{h0 + hh}_{ci}")
            pes = {}

            def s_chunk(kt, ci, hh, bt):
                # S.T matmul + bias add + exp for one (head, k-tile, q-chunk)
                p = ktsize(kt)
                q0, qn = QCHUNKS[ci]
                pr0 = hh * 64
                s_ps = ps_s.tile([128, 512], f32, tag="s_ps",
                                 name=f"sps_{h0 + hh}_{kt}_{ci}")
                nc.tensor.matmul(
                    s_ps[:p, :qn],
                    k_t[pr0:pr0 + 64, kt * 128: kt * 128 + p],
                    q_t[pr0:pr0 + 64, q0:q0 + qn],
                    start=True, stop=True,
                )
                pt = ppool.tile([128, 512], f32, tag="pt",
                                name=f"pt_{h0 + hh}_{kt}_{ci}", bufs=6)
                nc.vector.tensor_add(
                    pt[:p, :qn], s_ps[:p, :qn], bt[:p, q0:q0 + qn])
                pe = ppool.tile([128, 512], f32r, tag="pe",
                                name=f"pe_{h0 + hh}_{kt}_{ci}", bufs=18)
                nc.scalar.activation(pe[:p, :qn], pt[:p, :qn], AF.Exp)
                pes[(kt, hh, ci)] = pe

            def pv_chunk(kt, ci, hh):
                p = ktsize(kt)
                q0, qn = QCHUNKS[ci]
                nc.tensor.matmul(
                    o_ps[(hh, ci)][:, :qn],
                    v_sb[kt][:p, h0 + hh, :],
                    pes.pop((kt, hh, ci))[:p, :qn],
                    start=(kt == 0), stop=(kt == 8),
                )

            def unit(kt):
                # interleave the S pair (row-tiles 0/64 run concurrently on
                # PE) with the lagged PV pair so the PE stream stays dense
                p = ktsize(kt)
                bts = []
                for hh in (0, 1):
                    bt = bpool.tile([128, N], bf16, tag="bt",
                                    name=f"bt_{h0 + hh}_{kt}")
                    nc.sync.dma_start(bt[:p, :], bias_t[h0 + hh, kt, 0:p, :])
                    bts.append(bt)
                for ci in range(3):
                    s_chunk(kt, ci, 0, bts[0])
                    s_chunk(kt, ci, 1, bts[1])
                    if kt >= LAG:
                        pv_chunk(kt - LAG, ci, 0)
                        pv_chunk(kt - LAG, ci, 1)

            for kt in range(NKT):
                unit(kt)
            for kt in range(NKT - LAG, NKT):
                for ci in range(3):
                    pv_chunk(kt, ci, 0)
                    pv_chunk(kt, ci, 1)

            for ci, (q0, qn) in enumerate(QCHUNKS):
                for hh in (0, 1):
                    h = h0 + hh
                    rc = npool.tile([1, 512], f32, tag="rc",
                                    name=f"rc_{h}_{ci}")
                    nc.vector.reciprocal(rc[:, :qn], o_ps[(hh, ci)][64:65, :qn])
                    rcr = npool.tile([1, 512], f32r, tag="rcr",
                                     name=f"rcr_{h}_{ci}")
                    nc.scalar.copy(rcr[:, :qn], rc[:, :qn])
                    bc_ps = ps_s.tile([64, 512], f32, tag="s_ps",
                                      name=f"bcps_{h}_{ci}")
                    nc.tensor.matmul(bc_ps[:, :qn], ones64[:, :], rcr[:, :qn],
                                     start=True, stop=True)
                    bc = npool.tile([64, 512], f32, tag="bc",
                                    name=f"bc_{h}_{ci}")
                    nc.scalar.copy(bc[:, :qn], bc_ps[:, :qn])
                    nc.vector.tensor_mul(
                        o_all[pair][hh * 64:hh * 64 + 64, q0:q0 + qn],
                        o_ps[(hh, ci)][0:64, :qn],
                        bc[:, :qn],
                    )

    # ---------------- phase 3: output projection ----------------
    with tc.tile_pool(name="ops3", bufs=3, space="PSUM") as ps3, \
         tc.tile_pool(name="out_pool", bufs=3) as out_pool:
        for m in range(6):
            for (q0, qn) in QCHUNKS:
                ps = ps3.tile([128, 512], f32, tag="ps3", name=f"ps3_{m}_{q0}")
                for ct in range(6):
                    nc.tensor.matmul(
                        ps[:, :qn],
                        wproj_t[ct][:, m * 128:(m + 1) * 128],
                        o_all[ct][:, q0:q0 + qn],
                        start=(ct == 0), stop=(ct == 5),
                    )
                ot = out_pool.tile([128, 512], f32, tag="ot", name=f"ot_{m}_{q0}")
                nc.scalar.activation(ot[:, :qn], ps[:, :qn], AF.Identity,
                                     bias=projb_t[m])
                nc.sync.dma_start(outT[m * 128:(m + 1) * 128, q0:q0 + qn],
                                  ot[:, :qn])


def _host_prep(x, qkv_w, q_bias, v_bias, rpb_table, proj_w, proj_b,
               rel_pos_index):
    """Layout-only transforms; all FLOPs stay on device."""
    f = np.float32
    x = np.asarray(x, f)
    qkv_w = np.asarray(qkv_w, f)
    q_bias = np.asarray(q_bias, f)
    v_bias = np.asarray(v_bias, f)
    rpb_table = np.asarray(rpb_table, f)
    proj_w = np.asarray(proj_w, f)
    proj_b = np.asarray(proj_b, f)
    idx = np.asarray(rel_pos_index)

    # q/k weights: column blocks [q0 q1 | k0 k1 | q2 q3 | k2 k3 | ...],
    # q pre-scaled by 1/sqrt(D); bias realized via appended ones-row in xT.
    rows = []
    brows = []
    for p in range(6):
        rows.append(qkv_w[p * 128:(p + 1) * 128] * SCALE)
        brows.append(q_bias[p * 128:(p + 1) * 128] * SCALE)
        rows.append(qkv_w[C + p * 128: C + (p + 1) * 128])
        brows.append(np.zeros(128, f))
    wqk = np.concatenate(rows, axis=0)               # [1536, 768]
    wqk_bias = np.concatenate(brows, axis=0)         # [1536]
    wqk_aug = np.ascontiguousarray(wqk.T)            # [768, 1536]
    wqkb = np.ascontiguousarray(wqk_bias.reshape(12, 128).T)  # [128, 12]

    wv_aug = np.concatenate(
        [qkv_w[2 * C:3 * C].T, v_bias[None, :]], axis=0)          # [769, 768]

    import ml_dtypes
    rpb = rpb_table[idx]                              # [N, N, H] (q, k, h)
    biasT = np.ascontiguousarray(rpb.transpose(2, 1, 0))  # [H, k, q]
    bias_pad = np.zeros((H, NKT * 128, N), f)
    bias_pad[:, :N, :] = biasT
    bias_t = bias_pad.reshape(H, NKT, 128, N).astype(ml_dtypes.bfloat16)

    wproj = np.ascontiguousarray(proj_w.T)            # [768, 768]
    projb = np.ascontiguousarray(proj_b.reshape(C, 1))

    ones = np.ones((1, N), f)
    xT_aug = np.stack(
        [np.concatenate([np.ascontiguousarray(x[b].T), ones], axis=0)
         for b in range(B)], axis=0)                  # [B, 769, 1025]
    return xT_aug, wqk_aug, wqkb, wv_aug, bias_t, wproj, projb


_BUILT = {}


def _build():
    if "nc" in _BUILT:
        return _BUILT["nc"]
    from contextlib import ExitStack

    import concourse.mybir as mybir
    import concourse.tile as tile
    from concourse import bacc

    nc = bacc.Bacc("TRN2", target_bir_lowering=False, debug=False,
                   num_devices=B)
    f32 = mybir.dt.float32
    f32r = mybir.dt.float32r
    xT_aug = nc.dram_tensor("xT_aug", (769, N), f32r,
                            kind="ExternalInput").ap()
    wqk_aug = nc.dram_tensor("wqk_aug", (768, 1536), f32r,
                             kind="ExternalInput").ap()
    wqkb = nc.dram_tensor("wqkb", (128, 12), f32, kind="ExternalInput").ap()
    wv_aug = nc.dram_tensor("wv_aug", (769, 768), f32r,
                            kind="ExternalInput").ap()
    bias_t = nc.dram_tensor("bias_t", (H, NKT, 128, N), mybir.dt.bfloat16,
                            kind="ExternalInput").ap()
    wproj = nc.dram_tensor("wproj", (768, 768), f32r,
                           kind="ExternalInput").ap()
    projb = nc.dram_tensor("projb", (768, 1), f32, kind="ExternalInput").ap()
    outT = nc.dram_tensor("outT", (768, N), f32, kind="ExternalOutput").ap()

    with tile.TileContext(nc) as tc:
        with ExitStack() as ctx:
            _emit(ctx, tc, xT_aug, wqk_aug, wqkb, wv_aug, bias_t, wproj, projb, outT)
    nc.compile()
    _BUILT["nc"] = nc
    return nc


def kernel(x, qkv_w, q_bias, v_bias, rpb_table, proj_w, proj_b,
           rel_pos_index):
    from concourse.bass_utils import run_bass_kernel_spmd

    xT_aug, wqk_aug, wqkb, wv_aug, bias_t, wproj, projb = _host_prep(
        x, qkv_w, q_bias, v_bias, rpb_table, proj_w, proj_b, rel_pos_index)

    nc = _build()
    shared = {
        "wqk_aug": wqk_aug, "wqkb": wqkb, "wv_aug": wv_aug, "bias_t": bias_t,
        "wproj": wproj, "projb": projb,
    }
    in_maps = [dict(shared, xT_aug=np.ascontiguousarray(xT_aug[b]))
               for b in range(B)]
    res = run_bass_kernel_spmd(nc, in_maps, core_ids=list(range(B)))
    out = np.stack([res.results[b]["outT"].T for b in range(B)], axis=0)
    return out.astype(np.float32)


# revision 15
# speedup vs baseline: 1.0014x; 1.0014x over previous
"""Trainium2 Bass kernel for Swin-style windowed attention w/ relative position bias.

Problem: x[8, 1025, 768], 12 heads, head_dim 64, rel-pos bias table gathered
by a constant index matrix. Sharding: pure data-parallel — one batch element
per NeuronCore (8 cores).

Per-core dataflow (all matmuls f32r on PE; S kept transposed so softmax
normalize and P@V need no on-chip transposes; q padded 1025->1028 so all
matmul chunks are PSUM-bank aligned and have even width):
  xT_aug [769, 1028]   (x[b].T plus a ones-row that realizes the v bias add)
  qkT = wqk_aug.T @ xT_aug      -> [1536, 1028]  (q/k per head, channels-major,
                                   head pairs share a 128-partition tile)
  v   = xT_aug.T @ wv_aug       -> [1025, 768]   (tokens-major, + ones cols)
  S.T[k,q] = k_h @ q_h.T        (K=64; head pairs run as concurrent row-tiles)
  P.T = exp(S.T + biasT)        (bias streamed bf16, DVE add + one wide ACT exp)
  [O.T; rowsum] = [v_h|1].T @ P.T   (ones col gives softmax denominators)
  1/rowsum = exp(-ln(rowsum))   (two ACT ops; keeps the slow DVE reciprocal off
                                 the critical path)
  O.T *= (1/rowsum) broadcast   (K=1 ones matmul broadcast + DVE mult)
  outT = wproj.T @ O_all + proj_b   -> [768, 1025], host transposes back
"""

import sys

import numpy as np

for _p in ("/opt/trn_rl_repo",):
    if _p not in sys.path:
        sys.path.insert(0, _p)

B = 8
N = 1025
NP = 1028        # q padded to 2 full banks + one 4-wide tail chunk
C = 768
H = 12
D = 64
SCALE = D ** -0.5
NKT = 9          # k tiles of 128 (8 full + 1)
QCHUNKS = [(0, 512), (512, 512), (1024, 4)]
NCHUNKS_V = [(0, 512), (512, 256)]


def _emit(ctx, tc, xT_aug, wqk_aug, wqkb, wv_aug, bias_t, wproj, projb, outT):
    import concourse.mybir as mybir

    nc = tc.nc
    f32 = mybir.dt.float32
    f32r = mybir.dt.float32r
    bf16 = mybir.dt.bfloat16
    AF = mybir.ActivationFunctionType

    def ktsize(kt):
        return 128 if kt < 8 else 1

    lp = nc.allow_low_precision(
        reason="float32r is fp32-width storage; PE rounding only")
    lp.__enter__()
    ctx.callback(lambda: lp.__exit__(None, None, None))

    # Long-lived pools first (pool release must be LIFO / stack ordered).
    qk_pool = ctx.enter_context(tc.tile_pool(name="qk_pool", bufs=1))
    v_pool = ctx.enter_context(tc.tile_pool(name="v_pool", bufs=1))
    qk_sb = []
    for m in range(12):
        t = qk_pool.tile([128, NP], f32r, tag=f"qk{m}", name=f"qk{m}")
        qk_sb.append(t)
    v_sb = []
    for kt in range(NKT):
        t = v_pool.tile([128, H, D + 1], f32r, tag=f"v{kt}", name=f"v{kt}")
        v_sb.append(t)
        nc.vector.memset(t[:, :, D:D + 1].bitcast(f32), 1.0)
        nc.scalar.copy(t[:, :, D:D + 1], t[:, :, D:D + 1].bitcast(f32))

    # bias stream pool before phase-1 temporaries: its addresses must not
    # overlap phase-1 tiles so the DMAs can prefetch during the projections.
    bpool = ctx.enter_context(tc.tile_pool(name="bpool", bufs=6))

    # ---------------- phase 1: load weights + x, QKV projections -------------
    xpool = tc.alloc_tile_pool(name="xpool", bufs=1)
    wvpool = tc.alloc_tile_pool(name="wvpool", bufs=1)
    wqkpool = tc.alloc_tile_pool(name="wqkpool", bufs=1)

    wqk_t, wv_t, xT_t = [], [], []
    for ct in range(7):
        p = 128 if ct < 6 else 1
        xt = xpool.tile([p, NP], f32r, tag=f"xT{ct}", name=f"xT{ct}")
        nc.sync.dma_start(xt[:, :], xT_aug[ct * 128: ct * 128 + p, :])
        xT_t.append(xt)
        w2 = wvpool.tile([p, 768], f32r, tag=f"wv{ct}", name=f"wv{ct}")
        nc.sync.dma_start(w2[:, :], wv_aug[ct * 128: ct * 128 + p, :])
        wv_t.append(w2)
        if ct < 6:
            w1 = wqkpool.tile([p, 1536], f32r, tag=f"wqk{ct}",
                              name=f"wqk{ct}")
            nc.sync.dma_start(w1[:, :], wqk_aug[ct * 128:(ct + 1) * 128, :])
            wqk_t.append(w1)
    wqkb_t = wqkpool.tile([128, 12], f32, tag="wqkb", name="wqkb")
    nc.sync.dma_start(wqkb_t[:, :], wqkb[:, :])

    with tc.tile_pool(name="ps1", bufs=4, space="PSUM") as ps1, \
         tc.tile_pool(name="ps1v", bufs=2, space="PSUM") as ps1v:
        # q/k: qkT[m-tile, n] = sum_ct wqk[ct, m-tile].T @ xT[ct, n]
        for m in range(12):
            for (q0, qn) in QCHUNKS:
                ps = ps1.tile([128, 512], f32, tag="ps1t", name=f"ps1_{m}_{q0}")
                for ct in range(6):
                    nc.tensor.matmul(
                        ps[:, :qn],
                        wqk_t[ct][:, m * 128:(m + 1) * 128],
                        xT_t[ct][:, q0:q0 + qn],
                        start=(ct == 0), stop=(ct == 5),
                    )
                nc.scalar.activation(qk_sb[m][:, q0:q0 + qn], ps[:, :qn],
                                     AF.Identity, bias=wqkb_t[:, m:m + 1])
        # v: v[n-tile, c'] = sum_ct xT[ct, n-tile].T @ wv[ct, c']
        for kt in range(NKT):
            p = ktsize(kt)
            ps = ps1v.tile([128, 768], f32, tag="ps1v", name=f"ps1v_{kt}")
            for (c0, cn) in NCHUNKS_V:
                for ct in range(7):
                    nc.tensor.matmul(
                        ps[:p, c0:c0 + cn],
                        xT_t[ct][:, kt * 128: kt * 128 + p],
                        wv_t[ct][:, c0:c0 + cn],
                        start=(ct == 0), stop=(ct == 6),
                    )
            nc.scalar.copy(
                v_sb[kt][:p, :, 0:D],
                ps[:p, :].rearrange("p (h d) -> p h d", h=H),
            )

    wqkpool.release()
    wvpool.release()
    xpool.release()

    # ---------------- phase 2: attention, one head pair at a time ------------
    ppool = ctx.enter_context(tc.tile_pool(name="ppool", bufs=6))
    npool = ctx.enter_context(tc.tile_pool(name="npool", bufs=3))
    cpool = ctx.enter_context(tc.tile_pool(name="cpool", bufs=1))
    ones64 = cpool.tile([1, 64], f32r)
    nc.vector.memset(ones64[:, :].bitcast(f32), 1.0)
    nc.scalar.copy(ones64[:, :], ones64[:, :].bitcast(f32))
    opool = ctx.enter_context(tc.tile_pool(name="opool", bufs=1))
    o_all = []
    for m in range(6):
        t = opool.tile([128, NP], f32r, tag=f"oall{m}", name=f"oall{m}")
        o_all.append(t)

    wp_pool = ctx.enter_context(tc.tile_pool(name="wp_pool", bufs=1))
    wproj_t = []
    projb_t = []
    for ct in range(6):
        t = wp_pool.tile([128, 768], f32r, tag=f"wproj{ct}",
                         name=f"wproj{ct}")
        nc.sync.dma_start(t[:, :], wproj[ct * 128:(ct + 1) * 128, :])
        wproj_t.append(t)
        tb = wp_pool.tile([128, 1], f32, tag=f"projb{ct}", name=f"projb{ct}")
        nc.sync.dma_start(tb[:, :], projb[ct * 128:(ct + 1) * 128, :])
        projb_t.append(tb)

    LAG = 2
    pending_tails = []

    with tc.tile_pool(name="ps_s", bufs=2, space="PSUM") as ps_s, \
         tc.tile_pool(name="ps_o", bufs=6, space="PSUM") as ps_o:

        def make_tail(pair, hh, ci, o_ps_tile):
            h = 2 * pair + hh

            def tail():
                q0, qn = QCHUNKS[ci]
                lns = npool.tile([1, 512], f32, tag="lns",
                                 name=f"lns_{h}_{ci}")
                nc.scalar.activation(lns[:, :qn], o_ps_tile[64:65, :qn],
                                     AF.Ln)
                rcr = npool.tile([1, 512], f32r, tag="rcr",
                                 name=f"rcr_{h}_{ci}")
                nc.scalar.activation(rcr[:, :qn], lns[:, :qn], AF.Exp,
                                     scale=-1.0)
                bc_ps = ps_s.tile([64, 512], f32, tag="s_ps",
                                  name=f"bcps_{h}_{ci}")
                nc.tensor.matmul(bc_ps[:, :qn], ones64[:, :], rcr[:, :qn],
                                 start=True, stop=True)
                bc = npool.tile([64, 512], f32, tag="bc", name=f"bc_{h}_{ci}")
                nc.scalar.copy(bc[:, :qn], bc_ps[:, :qn])
                nc.vector.tensor_mul(
                    o_all[pair][hh * 64:hh * 64 + 64, q0:q0 + qn],
                    o_ps_tile[0:64, :qn],
                    bc[:, :qn],
                )

            return tail

        for pair in range(6):
            h0 = 2 * pair
            q_t = qk_sb[2 * pair]
            k_t = qk_sb[2 * pair + 1]
            o_ps = {}
            for hh in (0, 1):
                for ci in range(3):
                    o_ps[(hh, ci)] = ps_o.tile(
                        [65, 512], f32, tag="o_ps", name=f"ops_{h0 + hh}_{ci}")
            pes = {}

            def s_unit(kt):
                # S.T matmuls (head pair = concurrent PE row-tiles), bias
                # add, and one wide exp per head.
                p = ktsize(kt)
                bts = []
                for hh in (0, 1):
                    bt = bpool.tile([128, NP], bf16, tag="bt",
                                    name=f"bt_{h0 + hh}_{kt}")
                    nc.sync.dma_start(bt[:p, :], bias_t[h0 + hh, kt, 0:p, :])
                    bts.append(bt)
                pts = []
                for hh in (0, 1):
                    pt = ppool.tile([128, NP], f32, tag="pt",
                                    name=f"pt_{h0 + hh}_{kt}", bufs=4)
                    pts.append(pt)
                for ci, (q0, qn) in enumerate(QCHUNKS):
                    for hh in (0, 1):
                        pr0 = hh * 64
                        s_ps = ps_s.tile([128, 512], f32, tag="s_ps",
                                         name=f"sps_{h0 + hh}_{kt}_{ci}")
                        nc.tensor.matmul(
                            s_ps[:p, :qn],
                            k_t[pr0:pr0 + 64, kt * 128: kt * 128 + p],
                            q_t[pr0:pr0 + 64, q0:q0 + qn],
                            start=True, stop=True,
                        )
                        nc.vector.tensor_add(
                            pts[hh][:p, q0:q0 + qn], s_ps[:p, :qn],
                            bts[hh][:p, q0:q0 + qn])
                for hh in (0, 1):
                    pe = ppool.tile([128, NP], f32r, tag="pe",
                                    name=f"pe_{h0 + hh}_{kt}", bufs=6)
                    nc.scalar.activation(pe[:p, :], pts[hh][:p, :], AF.Exp)
                    pes[(kt, hh)] = pe

            def pv_unit(kt):
                p = ktsize(kt)
                for ci, (q0, qn) in enumerate(QCHUNKS):
                    for hh in (0, 1):
                        nc.tensor.matmul(
                            o_ps[(hh, ci)][:, :qn],
                            v_sb[kt][:p, h0 + hh, :],
                            pes[(kt, hh)][:p, q0:q0 + qn],
                            start=(kt == 0), stop=(kt == 8),
                        )
                for hh in (0, 1):
                    pes.pop((kt, hh))

            for kt in range(NKT):
                s_unit(kt)
                if kt >= LAG:
                    pv_unit(kt - LAG)
                # spread the previous pair's normalize tail across this
                # pair's units so PE never sees a long dependency stall
                if pending_tails:
                    pending_tails.pop(0)()
            for kt in range(NKT - LAG, NKT):
                pv_unit(kt)

            for ci in range(3):
                for hh in (0, 1):
                    pending_tails.append(
                        make_tail(pair, hh, ci, o_ps[(hh, ci)]))

        for t in pending_tails:
            t()

    # ---------------- phase 3: output projection ----------------
    with tc.tile_pool(name="ops3", bufs=3, space="PSUM") as ps3, \
         tc.tile_pool(name="out_pool", bufs=3) as out_pool:
        for m in range(6):
            for (q0, qn) in QCHUNKS:
                ps = ps3.tile([128, 512], f32, tag="ps3", name=f"ps3_{m}_{q0}")
                for ct in range(6):
                    nc.tensor.matmul(
                        ps[:, :qn],
                        wproj_t[ct][:, m * 128:(m + 1) * 128],
                        o_all[ct][:, q0:q0 + qn],
                        start=(ct == 0), stop=(ct == 5),
                    )
                wn = min(qn, N - q0)
                ot = out_pool.tile([128, 512], f32, tag="ot",
                                   name=f"ot_{m}_{q0}")
                nc.scalar.activation(ot[:, :wn], ps[:, :wn], AF.Identity,
                                     bias=projb_t[m])
                nc.sync.dma_start(outT[m * 128:(m + 1) * 128, q0:q0 + wn],
                                  ot[:, :wn])


def _host_prep(x, qkv_w, q_bias, v_bias, rpb_table, proj_w, proj_b,
               rel_pos_index):
    """Layout-only transforms; all FLOPs stay on device."""
    import ml_dtypes
    f = np.float32
    x = np.asarray(x, f)
    qkv_w = np.asarray(qkv_w, f)
    q_bias = np.asarray(q_bias, f)
    v_bias = np.asarray(v_bias, f)
    rpb_table = np.asarray(rpb_table, f)
    proj_w = np.asarray(proj_w, f)
    proj_b = np.asarray(proj_b, f)
    idx = np.asarray(rel_pos_index)

    # q/k weights: column blocks [q0 q1 | k0 k1 | q2 q3 | k2 k3 | ...],
    # q pre-scaled by 1/sqrt(D); q/k biases added at PSUM evacuation.
    rows = []
    brows = []
    for p in range(6):
        rows.append(qkv_w[p * 128:(p + 1) * 128] * SCALE)
        brows.append(q_bias[p * 128:(p + 1) * 128] * SCALE)
        rows.append(qkv_w[C + p * 128: C + (p + 1) * 128])
        brows.append(np.zeros(128, f))
    wqk = np.concatenate(rows, axis=0)               # [1536, 768]
    wqk_bias = np.concatenate(brows, axis=0)         # [1536]
    wqk_aug = np.ascontiguousarray(wqk.T)            # [768, 1536]
    wqkb = np.ascontiguousarray(wqk_bias.reshape(12, 128).T)  # [128, 12]

    wv_aug = np.concatenate(
        [qkv_w[2 * C:3 * C].T, v_bias[None, :]], axis=0)      # [769, 768]

    rpb = rpb_table[idx]                              # [N, N, H] (q, k, h)
    biasT = np.ascontiguousarray(rpb.transpose(2, 1, 0))  # [H, k, q]
    bias_pad = np.zeros((H, NKT * 128, NP), f)
    bias_pad[:, :N, :N] = biasT
    bias_t = bias_pad.reshape(H, NKT, 128, NP).astype(ml_dtypes.bfloat16)

    wproj = np.ascontiguousarray(proj_w.T)            # [768, 768]
    projb = np.ascontiguousarray(proj_b.reshape(C, 1))

    xT_aug = np.zeros((B, 769, NP), f)
    for b in range(B):
        xT_aug[b, :C, :N] = x[b].T
    xT_aug[:, C, :] = 1.0                             # bias row (ones)
    return xT_aug, wqk_aug, wqkb, wv_aug, bias_t, wproj, projb


_BUILT = {}


def _build():
    if "nc" in _BUILT:
        return _BUILT["nc"]
    from contextlib import ExitStack

    import concourse.mybir as mybir
    import concourse.tile as tile
    from concourse import bacc

    nc = bacc.Bacc("TRN2", target_bir_lowering=False, debug=False,
                   num_devices=B)
    f32 = mybir.dt.float32
    f32r = mybir.dt.float32r
    xT_aug = nc.dram_tensor("xT_aug", (769, NP), f32r,
                            kind="ExternalInput").ap()
    wqk_aug = nc.dram_tensor("wqk_aug", (768, 1536), f32r,
                             kind="ExternalInput").ap()
    wqkb = nc.dram_tensor("wqkb", (128, 12), f32, kind="ExternalInput").ap()
    wv_aug = nc.dram_tensor("wv_aug", (769, 768), f32r,
                            kind="ExternalInput").ap()
    bias_t = nc.dram_tensor("bias_t", (H, NKT, 128, NP), mybir.dt.bfloat16,
                            kind="ExternalInput").ap()
    wproj = nc.dram_tensor("wproj", (768, 768), f32r,
                           kind="ExternalInput").ap()
    projb = nc.dram_tensor("projb", (768, 1), f32, kind="ExternalInput").ap()
    outT = nc.dram_tensor("outT", (768, N), f32, kind="ExternalOutput").ap()

    with tile.TileContext(nc) as tc:
        with ExitStack() as ctx:
            _emit(ctx, tc, xT_aug, wqk_aug, wqkb, wv_aug, bias_t, wproj,
                  projb, outT)
    nc.compile()
    _BUILT["nc"] = nc
    return nc


def kernel(x, qkv_w, q_bias, v_bias, rpb_table, proj_w, proj_b,
           rel_pos_index):
    from concourse.bass_utils import run_bass_kernel_spmd

    xT_aug, wqk_aug, wqkb, wv_aug, bias_t, wproj, projb = _host_prep(
        x, qkv_w, q_bias, v_bias, rpb_table, proj_w, proj_b, rel_pos_index)

    nc = _build()
    shared = {
        "wqk_aug": wqk_aug, "wqkb": wqkb, "wv_aug": wv_aug, "bias_t": bias_t,
        "wproj": wproj, "projb": projb,
    }
    in_maps = [dict(shared, xT_aug=np.ascontiguousarray(xT_aug[b]))
               for b in range(B)]
    res = run_bass_kernel_spmd(nc, in_maps, core_ids=list(range(B)))
    out = np.stack([res.results[b]["outT"].T for b in range(B)], axis=0)
    return out.astype(np.float32)


# revision 18
# speedup vs baseline: 1.0930x; 1.0915x over previous
"""Trainium2 Bass kernel for Swin-style windowed attention w/ relative position bias.

Problem: x[8, 1025, 768], 12 heads, head_dim 64, rel-pos bias table gathered
by a constant index matrix. Sharding: pure data-parallel — one batch element
per NeuronCore (8 cores).

Per-core dataflow (all matmuls f32r on PE; S kept transposed so softmax
normalize and P@V need no on-chip transposes; q padded 1025->1028 so all
matmul chunks are PSUM-bank aligned and have even width):
  xT_aug [769, 1028]   (x[b].T plus a ones-row that realizes the v bias add)
  qkT = wqk_aug.T @ xT_aug      -> [1536, 1028]  (q/k per head, channels-major,
                                   head pairs share a 128-partition tile)
  v   = xT_aug.T @ wv_aug       -> [1025, 768]   (tokens-major, + ones cols)
  S.T[k,q] = k_h @ q_h.T        (K=64; head pairs run as concurrent row-tiles)
  P.T = exp(S.T + biasT)        (bias streamed bf16, DVE add + one wide ACT exp)
  [O.T; rowsum] = [v_h|1].T @ P.T   (ones col gives softmax denominators)
  1/rowsum = exp(-ln(rowsum))   (two ACT ops; keeps the slow DVE reciprocal off
                                 the critical path)
  O.T *= (1/rowsum) broadcast   (K=1 ones matmul broadcast + DVE mult)
  outT = wproj.T @ O_all + proj_b   -> [768, 1025], host transposes back
"""

import sys

import numpy as np

for _p in ("/opt/trn_rl_repo",):
    if _p not in sys.path:
        sys.path.insert(0, _p)

B = 8
N = 1025
NP = 1028        # q padded to 2 full banks + one 4-wide tail chunk
C = 768
H = 12
D = 64
SCALE = D ** -0.5
NKT = 9          # k tiles of 128 (8 full + 1)
QCHUNKS = [(0, 512), (512, 512), (1024, 4)]
NCHUNKS_V = [(0, 512), (512, 256)]


def _emit(ctx, tc, xT_aug, wqk_aug, wqkb, wv_aug, bias_t, wproj, projb, outT):
    import concourse.mybir as mybir

    nc = tc.nc
    f32 = mybir.dt.float32
    f32r = mybir.dt.float32r
    bf16 = mybir.dt.bfloat16
    AF = mybir.ActivationFunctionType

    def ktsize(kt):
        return 128 if kt < 8 else 1

    lp = nc.allow_low_precision(
        reason="float32r is fp32-width storage; PE rounding only")
    lp.__enter__()
    ctx.callback(lambda: lp.__exit__(None, None, None))

    # Long-lived pools first (pool release must be LIFO / stack ordered).
    qk_pool = ctx.enter_context(tc.tile_pool(name="qk_pool", bufs=1))
    v_pool = ctx.enter_context(tc.tile_pool(name="v_pool", bufs=1))
    qk_sb = []
    for m in range(12):
        t = qk_pool.tile([128, NP], f32r, tag=f"qk{m}", name=f"qk{m}")
        qk_sb.append(t)
    v_sb = []
    for kt in range(NKT):
        t = v_pool.tile([128, H, D + 1], f32r, tag=f"v{kt}", name=f"v{kt}")
        v_sb.append(t)
        nc.vector.memset(t[:, :, D:D + 1].bitcast(f32), 1.0)
        nc.scalar.copy(t[:, :, D:D + 1], t[:, :, D:D + 1].bitcast(f32))

    # bias stream pool before phase-1 temporaries: its addresses must not
    # overlap phase-1 tiles so the DMAs can prefetch during the projections.
    bpool = ctx.enter_context(tc.tile_pool(name="bpool", bufs=6))

    # ---------------- phase 1: load weights + x, QKV projections -------------
    xpool = tc.alloc_tile_pool(name="xpool", bufs=1)
    wvpool = tc.alloc_tile_pool(name="wvpool", bufs=1)
    wqkpool = tc.alloc_tile_pool(name="wqkpool", bufs=1)

    wqk_t, wv_t, xT_t = [], [], []
    for ct in range(7):
        p = 128 if ct < 6 else 1
        xt = xpool.tile([p, NP], f32r, tag=f"xT{ct}", name=f"xT{ct}")
        nc.sync.dma_start(xt[:, :], xT_aug[ct * 128: ct * 128 + p, :])
        xT_t.append(xt)
        w2 = wvpool.tile([p, 768], f32r, tag=f"wv{ct}", name=f"wv{ct}")
        nc.sync.dma_start(w2[:, :], wv_aug[ct * 128: ct * 128 + p, :])
        wv_t.append(w2)
        if ct < 6:
            w1 = wqkpool.tile([p, 1536], f32r, tag=f"wqk{ct}",
                              name=f"wqk{ct}")
            nc.sync.dma_start(w1[:, :], wqk_aug[ct * 128:(ct + 1) * 128, :])
            wqk_t.append(w1)
    wqkb_t = wqkpool.tile([128, 12], f32, tag="wqkb", name="wqkb")
    nc.sync.dma_start(wqkb_t[:, :], wqkb[:, :])

    with tc.tile_pool(name="ps1", bufs=4, space="PSUM") as ps1, \
         tc.tile_pool(name="ps1v", bufs=2, space="PSUM") as ps1v:
        # q/k: qkT[m-tile, n] = sum_ct wqk[ct, m-tile].T @ xT[ct, n]
        for m in range(12):
            for (q0, qn) in QCHUNKS:
                ps = ps1.tile([128, 512], f32, tag="ps1t", name=f"ps1_{m}_{q0}")
                for ct in range(6):
                    nc.tensor.matmul(
                        ps[:, :qn],
                        wqk_t[ct][:, m * 128:(m + 1) * 128],
                        xT_t[ct][:, q0:q0 + qn],
                        start=(ct == 0), stop=(ct == 5),
                    )
                nc.scalar.activation(qk_sb[m][:, q0:q0 + qn], ps[:, :qn],
                                     AF.Identity, bias=wqkb_t[:, m:m + 1])
        # v: v[n-tile, c'] = sum_ct xT[ct, n-tile].T @ wv[ct, c']
        for kt in range(NKT):
            p = ktsize(kt)
            ps = ps1v.tile([128, 768], f32, tag="ps1v", name=f"ps1v_{kt}")
            for (c0, cn) in NCHUNKS_V:
                for ct in range(7):
                    nc.tensor.matmul(
                        ps[:p, c0:c0 + cn],
                        xT_t[ct][:, kt * 128: kt * 128 + p],
                        wv_t[ct][:, c0:c0 + cn],
                        start=(ct == 0), stop=(ct == 6),
                    )
            nc.scalar.copy(
                v_sb[kt][:p, :, 0:D],
                ps[:p, :].rearrange("p (h d) -> p h d", h=H),
            )

    wqkpool.release()
    wvpool.release()
    xpool.release()

    # ---------------- phase 2: attention, one head pair at a time ------------
    ppool = ctx.enter_context(tc.tile_pool(name="ppool", bufs=6))
    npool = ctx.enter_context(tc.tile_pool(name="npool", bufs=3))
    cpool = ctx.enter_context(tc.tile_pool(name="cpool", bufs=1))
    ones64 = cpool.tile([1, 64], f32r)
    nc.vector.memset(ones64[:, :].bitcast(f32), 1.0)
    nc.scalar.copy(ones64[:, :], ones64[:, :].bitcast(f32))
    opool = ctx.enter_context(tc.tile_pool(name="opool", bufs=1))
    o_all = []
    for m in range(6):
        t = opool.tile([128, NP], f32r, tag=f"oall{m}", name=f"oall{m}")
        o_all.append(t)

    wp_pool = ctx.enter_context(tc.tile_pool(name="wp_pool", bufs=1))
    wproj_t = []
    projb_t = []
    for ct in range(6):
        t = wp_pool.tile([128, 768], f32r, tag=f"wproj{ct}",
                         name=f"wproj{ct}")
        nc.sync.dma_start(t[:, :], wproj[ct * 128:(ct + 1) * 128, :])
        wproj_t.append(t)
        tb = wp_pool.tile([128, 1], f32, tag=f"projb{ct}", name=f"projb{ct}")
        nc.sync.dma_start(tb[:, :], projb[ct * 128:(ct + 1) * 128, :])
        projb_t.append(tb)

    LAG = 3
    pending_tails = []

    with tc.tile_pool(name="ps_s", bufs=2, space="PSUM") as ps_s, \
         tc.tile_pool(name="ps_o", bufs=6, space="PSUM") as ps_o:

        def make_tail(pair, hh, ci, o_ps_tile):
            h = 2 * pair + hh

            def tail():
                q0, qn = QCHUNKS[ci]
                lns = npool.tile([1, 512], f32, tag="lns",
                                 name=f"lns_{h}_{ci}")
                nc.scalar.activation(lns[:, :qn], o_ps_tile[64:65, :qn],
                                     AF.Ln)
                rcr = npool.tile([1, 512], f32r, tag="rcr",
                                 name=f"rcr_{h}_{ci}")
                nc.scalar.activation(rcr[:, :qn], lns[:, :qn], AF.Exp,
                                     scale=-1.0)
                bc_ps = ps_s.tile([64, 512], f32, tag="s_ps",
                                  name=f"bcps_{h}_{ci}")
                nc.tensor.matmul(bc_ps[:, :qn], ones64[:, :], rcr[:, :qn],
                                 start=True, stop=True)
                bc = npool.tile([64, 512], f32, tag="bc", name=f"bc_{h}_{ci}")
                nc.vector.tensor_copy(bc[:, :qn], bc_ps[:, :qn])
                nc.vector.tensor_mul(
                    o_all[pair][hh * 64:hh * 64 + 64, q0:q0 + qn],
                    o_ps_tile[0:64, :qn],
                    bc[:, :qn],
                )

            return tail

        for pair in range(6):
            h0 = 2 * pair
            q_t = qk_sb[2 * pair]
            k_t = qk_sb[2 * pair + 1]
            o_ps = {}
            for hh in (0, 1):
                for ci in range(3):
                    o_ps[(hh, ci)] = ps_o.tile(
                        [65, 512], f32, tag="o_ps", name=f"ops_{h0 + hh}_{ci}")
            pes = {}

            def s_unit(kt):
                # S.T matmuls (head pair = concurrent PE row-tiles), bias
                # add, and one wide exp per head.
                p = ktsize(kt)
                bts = []
                for hh in (0, 1):
                    bt = bpool.tile([128, NP], bf16, tag="bt",
                                    name=f"bt_{h0 + hh}_{kt}")
                    nc.sync.dma_start(bt[:p, :], bias_t[h0 + hh, kt, 0:p, :])
                    bts.append(bt)
                pts = []
                for hh in (0, 1):
                    pt = ppool.tile([128, NP], f32, tag="pt",
                                    name=f"pt_{h0 + hh}_{kt}", bufs=4)
                    pts.append(pt)
                for ci, (q0, qn) in enumerate(QCHUNKS):
                    for hh in (0, 1):
                        pr0 = hh * 64
                        s_ps = ps_s.tile([128, 512], f32, tag="s_ps",
                                         name=f"sps_{h0 + hh}_{kt}_{ci}")
                        nc.tensor.matmul(
                            s_ps[:p, :qn],
                            k_t[pr0:pr0 + 64, kt * 128: kt * 128 + p],
                            q_t[pr0:pr0 + 64, q0:q0 + qn],
                            start=True, stop=True,
                        )
                        nc.vector.tensor_add(
                            pts[hh][:p, q0:q0 + qn],
                            s_ps[:p, :qn], bts[hh][:p, q0:q0 + qn])
                for hh in (0, 1):
                    pe = ppool.tile([128, NP], f32r, tag="pe",
                                    name=f"pe_{h0 + hh}_{kt}", bufs=8)
                    nc.scalar.activation(pe[:p, :], pts[hh][:p, :], AF.Exp)
                    pes[(kt, hh)] = pe

            def pv_unit(kt):
                p = ktsize(kt)
                for ci, (q0, qn) in enumerate(QCHUNKS):
                    for hh in (0, 1):
                        nc.tensor.matmul(
                            o_ps[(hh, ci)][:, :qn],
                            v_sb[kt][:p, h0 + hh, :],
                            pes[(kt, hh)][:p, q0:q0 + qn],
                            start=(kt == 0), stop=(kt == 8),
                        )
                for hh in (0, 1):
                    pes.pop((kt, hh))

            for kt in range(NKT):
                s_unit(kt)
                if kt >= LAG:
                    pv_unit(kt - LAG)
                # spread the previous pair's normalize tail across this
                # pair's first units (all six must land before this pair's
                # first PV needs the o_ps slots at kt=LAG)
                for _ in range(2):
                    if pending_tails:
                        pending_tails.pop(0)()
            for kt in range(NKT - LAG, NKT):
                pv_unit(kt)

            for ci in range(3):
                for hh in (0, 1):
                    pending_tails.append(
                        make_tail(pair, hh, ci, o_ps[(hh, ci)]))

        for t in pending_tails:
            t()

    # ---------------- phase 3: output projection ----------------
    with tc.tile_pool(name="ops3", bufs=3, space="PSUM") as ps3, \
         tc.tile_pool(name="out_pool", bufs=3) as out_pool:
        for m in range(6):
            for (q0, qn) in QCHUNKS:
                ps = ps3.tile([128, 512], f32, tag="ps3", name=f"ps3_{m}_{q0}")
                for ct in range(6):
                    nc.tensor.matmul(
                        ps[:, :qn],
                        wproj_t[ct][:, m * 128:(m + 1) * 128],
                        o_all[ct][:, q0:q0 + qn],
                        start=(ct == 0), stop=(ct == 5),
                    )
                wn = min(qn, N - q0)
                ot = out_pool.tile([128, 512], f32, tag="ot",
                                   name=f"ot_{m}_{q0}")
                nc.scalar.activation(ot[:, :wn], ps[:, :wn], AF.Identity,
                                     bias=projb_t[m])
                nc.sync.dma_start(outT[m * 128:(m + 1) * 128, q0:q0 + wn],
                                  ot[:, :wn])


def _host_prep(x, qkv_w, q_bias, v_bias, rpb_table, proj_w, proj_b,
               rel_pos_index):
    """Layout-only transforms; all FLOPs stay on device."""
    import ml_dtypes
    f = np.float32
    x = np.asarray(x, f)
    qkv_w = np.asarray(qkv_w, f)
    q_bias = np.asarray(q_bias, f)
    v_bias = np.asarray(v_bias, f)
    rpb_table = np.asarray(rpb_table, f)
    proj_w = np.asarray(proj_w, f)
    proj_b = np.asarray(proj_b, f)
    idx = np.asarray(rel_pos_index)

    # q/k weights: column blocks [q0 q1 | k0 k1 | q2 q3 | k2 k3 | ...],
    # q pre-scaled by 1/sqrt(D); q/k biases added at PSUM evacuation.
    rows = []
    brows = []
    for p in range(6):
        rows.append(qkv_w[p * 128:(p + 1) * 128] * SCALE)
        brows.append(q_bias[p * 128:(p + 1) * 128] * SCALE)
        rows.append(qkv_w[C + p * 128: C + (p + 1) * 128])
        brows.append(np.zeros(128, f))
    wqk = np.concatenate(rows, axis=0)               # [1536, 768]
    wqk_bias = np.concatenate(brows, axis=0)         # [1536]
    wqk_aug = np.ascontiguousarray(wqk.T)            # [768, 1536]
    wqkb = np.ascontiguousarray(wqk_bias.reshape(12, 128).T)  # [128, 12]

    wv_aug = np.concatenate(
        [qkv_w[2 * C:3 * C].T, v_bias[None, :]], axis=0)      # [769, 768]

    rpb = rpb_table[idx]                              # [N, N, H] (q, k, h)
    biasT = np.ascontiguousarray(rpb.transpose(2, 1, 0))  # [H, k, q]
    bias_pad = np.zeros((H, NKT * 128, NP), f)
    bias_pad[:, :N, :N] = biasT
    bias_t = bias_pad.reshape(H, NKT, 128, NP).astype(ml_dtypes.bfloat16)

    wproj = np.ascontiguousarray(proj_w.T)            # [768, 768]
    projb = np.ascontiguousarray(proj_b.reshape(C, 1))

    xT_aug = np.zeros((B, 769, NP), f)
    for b in range(B):
        xT_aug[b, :C, :N] = x[b].T
    xT_aug[:, C, :] = 1.0                             # bias row (ones)
    return xT_aug, wqk_aug, wqkb, wv_aug, bias_t, wproj, projb


_BUILT = {}


def _build():
    if "nc" in _BUILT:
        return _BUILT["nc"]
    from contextlib import ExitStack

    import concourse.mybir as mybir
    import concourse.tile as tile
    from concourse import bacc

    nc = bacc.Bacc("TRN2", target_bir_lowering=False, debug=False,
                   num_devices=B)
    f32 = mybir.dt.float32
    f32r = mybir.dt.float32r
    xT_aug = nc.dram_tensor("xT_aug", (769, NP), f32r,
                            kind="ExternalInput").ap()
    wqk_aug = nc.dram_tensor("wqk_aug", (768, 1536), f32r,
                             kind="ExternalInput").ap()
    wqkb = nc.dram_tensor("wqkb", (128, 12), f32, kind="ExternalInput").ap()
    wv_aug = nc.dram_tensor("wv_aug", (769, 768), f32r,
                            kind="ExternalInput").ap()
    bias_t = nc.dram_tensor("bias_t", (H, NKT, 128, NP), mybir.dt.bfloat16,
                            kind="ExternalInput").ap()
    wproj = nc.dram_tensor("wproj", (768, 768), f32r,
                           kind="ExternalInput").ap()
    projb = nc.dram_tensor("projb", (768, 1), f32, kind="ExternalInput").ap()
    outT = nc.dram_tensor("outT", (768, N), f32, kind="ExternalOutput").ap()

    with tile.TileContext(nc) as tc:
        with ExitStack() as ctx:
            _emit(ctx, tc, xT_aug, wqk_aug, wqkb, wv_aug, bias_t, wproj,
                  projb, outT)
    nc.compile()
    _BUILT["nc"] = nc
    return nc


def kernel(x, qkv_w, q_bias, v_bias, rpb_table, proj_w, proj_b,
           rel_pos_index):
    from concourse.bass_utils import run_bass_kernel_spmd

    xT_aug, wqk_aug, wqkb, wv_aug, bias_t, wproj, projb = _host_prep(
        x, qkv_w, q_bias, v_bias, rpb_table, proj_w, proj_b, rel_pos_index)

    nc = _build()
    shared = {
        "wqk_aug": wqk_aug, "wqkb": wqkb, "wv_aug": wv_aug, "bias_t": bias_t,
        "wproj": wproj, "projb": projb,
    }
    in_maps = [dict(shared, xT_aug=np.ascontiguousarray(xT_aug[b]))
               for b in range(B)]
    res = run_bass_kernel_spmd(nc, in_maps, core_ids=list(range(B)))
    out = np.stack([res.results[b]["outT"].T for b in range(B)], axis=0)
    return out.astype(np.float32)
